# revision 47
# baseline (speedup 1.0000x reference)
"""Distributed Trainium2 kernel for LoRA-fused GQA attention.

Sharding: 8 cores = 2 (batch) x 4 (head-pairs). Core 4*b+j computes batch b,
heads {2j, 2j+1} end-to-end (q-proj, RoPE, causal attention vs the shared
KV head, partial out-proj); the host sums the 4 partial out-projections per
batch. LoRA adapters are folded into effective projection weights on the host
(exact algebraic rewrite; the einsum's repeated head index is a diagonal).

v2: the PE p-state ramp only reaches full clock (2.4 GHz) after ~3us of
gap-free execution; any short stall drops it to ~2.0 GHz. This version keeps
the PE stream continuous: the PV matmul is flipped to produce enc^T [t, H+1]
with a ones-column appended to V so the softmax denominator arrives as a free
extra output column (no separate l-matmuls, no PSUM-serialized l tile, no
DRAM-broadcast normalize — just a per-partition reciprocal + tensor_scalar).
enc^T is PE-transposed back to [nh, t] for the out-projection. All matmuls
use uniform 128x128 stationary tiles.

Self-contained: hardcodes all shapes from the problem spec.
"""

import os

import numpy as np
import ml_dtypes

B, T, D, N, K, H, L = 2, 2048, 2048, 8, 1, 256, 16
LORA_SCALE = 16.0 / 16
BIG_NEG = -2.3819763e38
N_CORES = 8
P = 128
NH = N * H  # 2048
TJ = 512  # t-chunk width (free dim of logits tiles)
NTJ = T // TJ  # 4
NST = T // P  # 16 s-tiles
DCH = D // P  # 16 contraction chunks
HEADS_PER_CORE = 2
NHC = HEADS_PER_CORE * H // P  # 4 q/enc row-tiles of 128 per core

LAST_EXEC_TIME_NS = None

_BF16 = ml_dtypes.bfloat16


def _fold_weights(q_w, q_lora_a, q_lora_b, kv_w, kv_lora_a, kv_lora_b,
                  out_w, out_lora_a, out_lora_b):
    """Fold LoRA into effective dense weights (all float32 math)."""
    # q: [N,D,H] + a[N,D,L] @ diag_b[N,L,H] -> Wq_eff [D, N*H]
    bd = np.stack([q_lora_b[n, :, n, :] for n in range(N)])  # [N,L,H]
    wq = q_w + np.einsum("ndl,nlh->ndh", q_lora_a, bd) * LORA_SCALE
    wq_eff = wq.transpose(1, 0, 2).reshape(D, NH).astype(np.float32)
    # kv: [2,K,D,H], K=1
    kvb = kv_lora_b[:, 0, :, 0, :]  # [2,L,H]
    wkv = kv_w[:, 0] + np.einsum("idl,ilh->idh", kv_lora_a[:, 0], kvb) * LORA_SCALE
    wk_eff = wkv[0].astype(np.float32)  # [D,H]
    wv_eff = wkv[1].astype(np.float32)  # [D,H]
    # out: [N,H,D] + diag_a[N,H,L] @ b[N,L,D] -> Wo_eff [N*H, D]
    ad = np.stack([out_lora_a[n, :, n, :] for n in range(N)])  # [N,H,L]
    wo = out_w + np.einsum("nhl,nld->nhd", ad, out_lora_b) * LORA_SCALE
    wo_eff = wo.reshape(NH, D).astype(np.float32)
    return wq_eff, wk_eff, wv_eff, wo_eff


def _rope_tables(positions_b):
    """cos/sin [P, T] f32 for one batch row of positions."""
    freq_exp = (2.0 / H) * np.arange(H // 2, dtype=np.float32)
    timescale = (10000.0 ** freq_exp).astype(np.float32)  # [128]
    rad = positions_b.astype(np.float32)[None, :] / timescale[:, None]  # [128,T]
    return np.cos(rad).astype(np.float32), np.sin(rad).astype(np.float32)


def _classify_mask(attn_mask):
    """Tile-classify the (shared-program) mask.

    Returns (s_tiles, lo_cols, bias_list):
      s_tiles[tj]: ordered 128-wide s-tile indices to process for t-chunk tj
      lo_cols[(tj, st)]: leading all-false column count in the [128 s, TJ t]
        logits tile (those t-columns are skipped: exp never runs there and
        the flipped PV never reads them)
      bias_list: (tj, st, mc) 128-col sub-blocks needing an additive bias
    Inclusion is the union over both batches so the SPMD program is
    identical on every core; per-core bias data covers the rest.
    """
    m = np.asarray(attn_mask)[:, 0]  # [B, T(query), S(key)]
    s_tiles = []
    lo_cols = {}
    bias_list = []
    for tj in range(NTJ):
        tiles = []
        for st in range(NST):
            sk = slice(st * P, (st + 1) * P)
            subs = [m[:, tj * TJ + mc * P:tj * TJ + (mc + 1) * P, sk]
                    for mc in range(TJ // P)]
            if not any(s.any() for s in subs):
                continue
            tiles.append(st)
            lo = 0
            while lo < len(subs) and not subs[lo].any():
                lo += 1
            lo_cols[(tj, st)] = lo * P
            for mc in range(lo, len(subs)):
                if not subs[mc].all():
                    bias_list.append((tj, st, mc))
        s_tiles.append(tiles)
    return s_tiles, lo_cols, bias_list


def _build_program(s_tiles, lo_cols, bias_list):
    """Build + compile the single SPMD Bass program."""
    from contextlib import ExitStack

    import concourse.bass as bass  # noqa: F401
    import concourse.tile as tile
    from concourse import bacc, mybir

    bf16 = mybir.dt.bfloat16
    f32 = mybir.dt.float32
    AF = mybir.ActivationFunctionType

    nc = bacc.Bacc("TRN2", target_bir_lowering=False, debug=False,
                   num_devices=N_CORES)

    xT_e = nc.dram_tensor("xT", [DCH, P, T], bf16, kind="ExternalInput")
    wq_e = nc.dram_tensor("wq", [NHC, P, DCH * P], bf16, kind="ExternalInput")
    wk_e = nc.dram_tensor("wk", [2, P, DCH * P], bf16, kind="ExternalInput")
    wv_e = nc.dram_tensor("wv", [DCH, P, H], bf16, kind="ExternalInput")
    wo_e = nc.dram_tensor("wo", [DCH, P, NHC * P], bf16, kind="ExternalInput")
    csq_e = nc.dram_tensor("csq", [2, P, T], bf16, kind="ExternalInput")
    csk_e = nc.dram_tensor("csk", [2, P, T], bf16, kind="ExternalInput")
    id_e = nc.dram_tensor("ident", [P, P], bf16, kind="ExternalInput")
    nbias = max(1, len(bias_list))
    bias_e = nc.dram_tensor("bias", [nbias, P, P], bf16, kind="ExternalInput")
    out_e = nc.dram_tensor("out", [DCH, P, T], bf16, kind="ExternalOutput")

    bias_idx = {k: i for i, k in enumerate(bias_list)}

    with tile.TileContext(nc) as tc:
        with ExitStack() as ctx:
            sb = ctx.enter_context(tc.tile_pool(name="sb", bufs=1))
            ps = ctx.enter_context(tc.tile_pool(name="ps", bufs=1,
                                                space="PSUM"))

            warm_w = sb.tile([P, P], bf16, tag="warm", bufs=2, name="warmw")
            nc.vector.memset(warm_w[:, :], 0.0)
            warm_src = sb.tile([P, TJ], bf16, tag="warm2", bufs=1,
                               name="warms")
            nc.vector.memset(warm_src[:, :], 0.0)

            # ---- loads, in consumption order. wave1 of the q-projection
            # only uses head-block 0, so load wq0 + the first x chunks first
            # and start the real matmuls ~4us sooner.
            wq_sb = []
            for n in range(NHC):
                t_ = sb.tile([P, DCH * P], bf16, tag="wq", bufs=NHC,
                             name=f"wqs{n}")
                wq_sb.append(t_)
            xt = []
            for c in range(DCH):
                t_ = sb.tile([P, T], bf16, tag="xs", bufs=DCH, name=f"xt{c}")
                xt.append(t_)
            nc.sync.dma_start(wq_sb[0][:, :], wq_e.ap()[0])
            for c in range(2):
                nc.sync.dma_start(xt[c][:, :], xT_e.ap()[c])
            for n in range(1, NHC):
                nc.sync.dma_start(wq_sb[n][:, :], wq_e.ap()[n])
            wk_sb = []
            for j in range(2):
                t_ = sb.tile([P, DCH * P], bf16, tag="wk", bufs=2,
                             name=f"wks{j}")
                nc.sync.dma_start(t_[:, :], wk_e.ap()[j])
                wk_sb.append(t_)
            for c in range(2, DCH):
                nc.sync.dma_start(xt[c][:, :], xT_e.ap()[c])
            csq = []
            csk = []
            for i in range(2):
                t_ = sb.tile([P, T], bf16, tag="cs", bufs=4, name=f"csq{i}")
                nc.sync.dma_start(t_[:, :], csq_e.ap()[i])
                csq.append(t_)
            for i in range(2):
                t_ = sb.tile([P, T], bf16, tag="cs", bufs=4, name=f"csk{i}")
                nc.sync.dma_start(t_[:, :], csk_e.ap()[i])
                csk.append(t_)
            # v tiles carry a ones-column at [:, H] so the flipped PV matmul
            # emits the softmax denominator as output column H
            v_sb = []
            for st in range(NST):
                vt = sb.tile([P, H + 1], bf16, tag="v", bufs=NST,
                             name=f"v{st}")
                nc.vector.memset(vt[:, H:H + 1], 1.0)
                v_sb.append(vt)
            wv_sb = []
            for c in range(DCH):
                t_ = sb.tile([P, H], bf16, tag="wv", bufs=DCH, name=f"wv{c}")
                nc.sync.dma_start(t_[:, :], wv_e.ap()[c])
                wv_sb.append(t_)
            wo_sb = []
            for dt in range(DCH):
                t_ = sb.tile([P, NHC * P], bf16, tag="wo", bufs=DCH,
                             name=f"wos{dt}")
                nc.sync.dma_start(t_[:, :], wo_e.ap()[dt])
                wo_sb.append(t_)
            ident = sb.tile([P, P], bf16, tag="ident", bufs=1, name="ident")
            nc.sync.dma_start(ident[:, :], id_e.ap())
            bias_sb = {}
            for key in bias_list:
                tj, st, mc = key
                t_ = sb.tile([P, P], bf16, tag="bias", bufs=nbias,
                             name=f"bias{tj}_{st}_{mc}")
                nc.sync.dma_start(t_[:, :], bias_e.ap()[bias_idx[key]])
                bias_sb[key] = t_

            # PE warm-up during the initial DMA wait: throwaway matmuls with
            # the same 128x128 stationary shape as everything else, so the
            # p-state ramp opens before real work lands.
            def filler():
                wps = ps.tile([P, TJ], f32, tag="ptr", bufs=2, name="wps")
                nc.tensor.matmul(wps[:, :], warm_w[:, :], warm_src[:, :],
                                 start=True, stop=True)

            for i in range(24):
                filler()

            def rope_pair(dst0, dst1, src0, src1, cs):
                """dst0 = s0*cos - s1*sin ; dst1 = s1*cos + s0*sin."""
                cos_t, sin_t = cs
                a = sb.tile([P, T], bf16, tag="qk", bufs=10, name="ropeA")
                nc.vector.tensor_mul(a[:, :], src0[:, :], cos_t[:, :])
                bmul = sb.tile([P, T], bf16, tag="qk", bufs=10, name="ropeB")
                nc.vector.tensor_mul(bmul[:, :], src1[:, :], sin_t[:, :])
                c2 = sb.tile([P, T], bf16, tag="qk", bufs=10, name="ropeC")
                nc.vector.tensor_mul(c2[:, :], src1[:, :], cos_t[:, :])
                d2 = sb.tile([P, T], bf16, tag="qk", bufs=10, name="ropeD")
                nc.vector.tensor_mul(d2[:, :], src0[:, :], sin_t[:, :])
                nc.vector.tensor_sub(dst0[:, :], a[:, :], bmul[:, :])
                nc.vector.tensor_add(dst1[:, :], c2[:, :], d2[:, :])

            # ---- stage A: q projection (2 heads) + RoPE ----
            # psum group tags: "pse" x4 + "lg" x2 rotate through projections
            atags = [("pse", 2), ("pse", 2), ("lg", 4), ("lg", 4),
                     ("lg", 4), ("lg", 4)]
            qraw = [sb.tile([P, T], bf16, tag="qk", bufs=10, name=f"qraw{n}")
                    for n in range(NHC)]
            groups = [(n, t4) for n in range(NHC) for t4 in range(T // 512)]
            # wave 1: 4 open accumulation groups on the "pse" banks,
            # interleaved chunk-major so the PE consumes each x chunk the
            # moment its DMA lands; fillers (on "lg") pad the per-chunk PE
            # time to the DMA cadence.
            wave1 = groups[:4]
            pq1 = {}
            for (n, t4) in wave1:
                pq1[(n, t4)] = ps.tile([P, 512], f32, tag="lg", bufs=4,
                                       name="pq")
            for c in range(DCH):
                for (n, t4) in wave1:
                    nc.tensor.matmul(
                        pq1[(n, t4)][:, :],
                        wq_sb[n][:, c * P:(c + 1) * P],
                        xt[c][:, t4 * 512:(t4 + 1) * 512],
                        start=(c == 0), stop=(c == DCH - 1),
                    )
                filler()
                filler()
            for (n, t4) in wave1:
                nc.any.tensor_copy(qraw[n][:, t4 * 512:(t4 + 1) * 512],
                                   pq1[(n, t4)][:, :])
            # wave 2: x is resident by now; plain group-major
            gi = 0
            for (n, t4) in groups[4:]:
                tg, nb = atags[gi % len(atags)]
                gi += 1
                pq = ps.tile([P, 512], f32, tag=tg, bufs=nb, name="pq")
                for c in range(DCH):
                    nc.tensor.matmul(
                        pq[:, :],
                        wq_sb[n][:, c * P:(c + 1) * P],
                        xt[c][:, t4 * 512:(t4 + 1) * 512],
                        start=(c == 0), stop=(c == DCH - 1),
                    )
                nc.any.tensor_copy(qraw[n][:, t4 * 512:(t4 + 1) * 512],
                                   pq[:, :])
            q = []
            for hh in range(HEADS_PER_CORE):
                q0 = sb.tile([P, T], bf16, tag="qk", bufs=10, name=f"q{2*hh}")
                q1 = sb.tile([P, T], bf16, tag="qk", bufs=10, name=f"q{2*hh+1}")
                rope_pair(q0, q1, qraw[2 * hh], qraw[2 * hh + 1], csq)
                q.extend([q0, q1])

            # ---- stage B: k/v projection (replicated KV head) + RoPE ----
            kraw = []
            for j in range(2):
                kn = sb.tile([P, T], bf16, tag="qk", bufs=10, name=f"kraw{j}")
                for t4 in range(T // 512):
                    tg, nb = atags[gi % len(atags)]
                    gi += 1
                    pk = ps.tile([P, 512], f32, tag=tg, bufs=nb, name="pk")
                    for c in range(DCH):
                        nc.tensor.matmul(
                            pk[:, :],
                            wk_sb[j][:, c * P:(c + 1) * P],
                            xt[c][:, t4 * 512:(t4 + 1) * 512],
                            start=(c == 0), stop=(c == DCH - 1),
                        )
                    nc.any.tensor_copy(kn[:, t4 * 512:(t4 + 1) * 512], pk[:, :])
                kraw.append(kn)
            k0 = sb.tile([P, T], bf16, tag="qk", bufs=10, name="k0")
            k1 = sb.tile([P, T], bf16, tag="qk", bufs=10, name="k1")
            # k rope column-chunked: the first QK matmuls gate on the low
            # key columns, so finish those first
            kta = sb.tile([P, T], bf16, tag="qk", bufs=10, name="kta")
            ktb = sb.tile([P, T], bf16, tag="qk", bufs=10, name="ktb")
            ktc = sb.tile([P, T], bf16, tag="qk", bufs=10, name="ktc")
            ktd = sb.tile([P, T], bf16, tag="qk", bufs=10, name="ktd")
            for (x0, x1) in [(0, 512), (512, T)]:
                sl = slice(x0, x1)
                nc.vector.tensor_mul(kta[:, sl], kraw[0][:, sl], csk[0][:, sl])
                nc.vector.tensor_mul(ktb[:, sl], kraw[1][:, sl], csk[1][:, sl])
                nc.vector.tensor_sub(k0[:, sl], kta[:, sl], ktb[:, sl])
                nc.vector.tensor_mul(ktc[:, sl], kraw[1][:, sl], csk[0][:, sl])
                nc.vector.tensor_mul(ktd[:, sl], kraw[0][:, sl], csk[1][:, sl])
                nc.vector.tensor_add(k1[:, sl], ktc[:, sl], ktd[:, sl])

            for st in range(NST):
                tg, nb = atags[gi % len(atags)]
                gi += 1
                pv = ps.tile([P, H], f32, tag=tg, bufs=nb, name="pv")
                for c in range(DCH):
                    nc.tensor.matmul(
                        pv[:, :],
                        xt[c][:, st * P:(st + 1) * P],
                        wv_sb[c][:, :],
                        start=(c == 0), stop=(c == DCH - 1),
                    )
                # explicit DVE: ACT's in-order queue must stay clear for exp
                nc.vector.tensor_copy(v_sb[st][:, 0:H], pv[:, :])

            # ---- stage C: attention per (head, t-chunk) ----
            # QK unchanged: lg [128s, TJ t]. PV flipped: for each 128-wide
            # t-slice, enc_T[128t, H+1] = sum_st p[st][:, t-slice]^T @ [v|1].
            # Column H is the softmax denominator. Normalize with a
            # per-partition reciprocal; PE-transpose back to [nh, t] blocks
            # one t-chunk later; out-proj consumes the reassembled enc.
            enc = []
            for n in range(NHC):
                # tag "xs": reuse the xT slots (dead after the v projection)
                en = sb.tile([P, T], bf16, tag="xs", bufs=DCH, name=f"enc{n}")
                enc.append(en)
            # normalized enc_T slices [128t, H] per (hh, local ts); live
            # until transposed during the next t-chunk
            encnt = {}

            def qk_step(hh, tj, st):
                """Emit QK + bias + exp for one s-tile; return pv args."""
                tsl0 = tj * TJ
                ssl = slice(st * P, (st + 1) * P)
                lo = lo_cols[(tj, st)]
                lg = ps.tile([P, TJ], f32, tag="lg", bufs=4, name="lg")
                nc.tensor.matmul(lg[:, lo:], k0[:, ssl],
                                 q[2 * hh][:, tsl0 + lo:tsl0 + TJ],
                                 start=True, stop=False)
                nc.tensor.matmul(lg[:, lo:], k1[:, ssl],
                                 q[2 * hh + 1][:, tsl0 + lo:tsl0 + TJ],
                                 start=False, stop=True)
                for mc in range(lo // P, TJ // P):
                    key = (tj, st, mc)
                    if key in bias_sb:
                        msl = slice(mc * P, (mc + 1) * P)
                        nc.vector.tensor_add(lg[:, msl], lg[:, msl],
                                             bias_sb[key][:, :])
                p_sb = sb.tile([P, TJ], bf16, tag="p", bufs=17, name="psb")
                nc.scalar.activation(p_sb[:, lo:], lg[:, lo:], AF.Exp)
                return (st, lo, p_sb)

            def attention(hh, tj, act_jobs, pe_jobs=None, pe_gate=0,
                          pre=None, tail_jobs=None):
                """Two-phase unit: t-slices are processed in pairs so only
                2 encT banks are live at once (pse bufs=2), freeing lg to 4
                banks. Pair 1 replays the cached p tiles as a ready-PV
                backlog, so its new QK/exp latencies are fully hidden.

                pre: pv args for this unit's first tiles (QK/exp emitted by
                the previous unit). tail_jobs: closures emitting the NEXT
                unit's first QK steps, interleaved into this unit's final PV
                flush so the in-order PE never drains on the last exp."""
                tiles = s_tiles[tj]
                nts = TJ // P  # 4 local t-slices
                if not tiles:
                    for ts in range(nts):
                        en = sb.tile([P, H], bf16, tag="ent", bufs=8,
                                     name="encnt")
                        nc.vector.memset(en[:, :], 0.0)
                        encnt[(hh, tj * 4 + ts)] = en
                    return
                first_for = {}
                last_for = {}
                for st in tiles:
                    for ts in range(nts):
                        if (tj * 4 + ts) >= st:
                            first_for.setdefault(ts, st)
                            last_for[ts] = st
                alt = [0]
                p_cache = {}
                for (st, lo, p_sb) in (pre or []):
                    p_cache[st] = (lo, p_sb)
                tails = list(tail_jobs or [])

                def normalize(ts):
                    en = sb.tile([P, H], bf16, tag="ent", bufs=8,
                                 name="encnt")
                    if ts not in first_for:
                        nc.vector.memset(en[:, :], 0.0)
                        encnt[(hh, tj * 4 + ts)] = en
                        return
                    rec = sb.tile([P, 1], f32, tag="rec", bufs=4, name="rec")
                    nc.vector.reciprocal_approx_fast(
                        out=rec[:, :], in_=encT[ts][:, H:H + 1])
                    nc.vector.tensor_scalar_mul(en[:, :],
                                                encT[ts][:, 0:H], rec[:, :])
                    encnt[(hh, tj * 4 + ts)] = en

                for pair in range(2):
                    ts_list = [2 * pair, 2 * pair + 1]
                    ts_exist = [ts for ts in ts_list if ts in first_for]
                    if not ts_exist:
                        for ts in ts_list:
                            normalize(ts)
                        continue
                    encT = {}
                    for ts in ts_exist:
                        encT[ts] = ps.tile([P, TJ], f32, tag="pse", bufs=2,
                                           name="encT")

                    def emit_pv(st, lo, p_sb):
                        for ts in ts_exist:
                            tt = tj * 4 + ts
                            if tt < st:
                                continue
                            nc.tensor.matmul(
                                encT[ts][:, 0:H + 1],
                                p_sb[:, ts * P:(ts + 1) * P],
                                v_sb[st][:, :],
                                start=(st == first_for[ts]),
                                stop=(st == last_for[ts]),
                            )

                    tt_hi = tj * 4 + ts_exist[-1]
                    needed = [st for st in tiles if st <= tt_hi]
                    backlog = [st for st in needed if st in p_cache]
                    new_tiles = [st for st in needed if st not in p_cache]
                    pv_q = []
                    for idx, st in enumerate(new_tiles):
                        p_cache[st] = qk_step(hh, tj, st)[1:]
                        # pop AFTER qk_step: the bias add enters the DVE
                        # queue ahead of the po copy, and the po matmuls
                        # fill the exp wait
                        if pe_jobs and (pair == 1 or idx >= pe_gate):
                            alt[0] += 1
                            pe_jobs.pop(0)(use_act=(pair == 1
                                                    and alt[0] % 2 == 0))
                        if act_jobs:
                            act_jobs.pop(0)()  # deferred ACT enc copy
                        if backlog:  # ready PVs hide the new exp latency
                            bst = backlog.pop(0)
                            emit_pv(bst, *p_cache[bst])
                        pv_q.append(st)
                        if len(pv_q) > 2:
                            bst = pv_q.pop(0)
                            emit_pv(bst, *p_cache[bst])
                    for st in backlog:
                        if pe_jobs and pair == 1:
                            alt[0] += 1
                            pe_jobs.pop(0)(use_act=(alt[0] % 2 == 0))
                        emit_pv(st, *p_cache[st])
                    last_flush = pair == 1 or not any(
                        ts in first_for for ts in (2, 3))
                    for st in pv_q:
                        if last_flush and tails:
                            tails.pop(0)()  # next unit's QK fills exp wait
                        emit_pv(st, *p_cache[st])
                    for ts in ts_list:
                        normalize(ts)
                for job in tails:
                    job()

            def transposes(tj):
                """PE-transpose enc_T(tj) into [nh, t] blocks; returns the
                deferred ACT copy closures (one per nh block)."""
                jobs = []
                for pair in range(NHC // 2):
                    ptr = ps.tile([P, 2 * TJ], bf16, tag="ptr", bufs=2,
                                  name="ptr")
                    for sub in range(2):
                        cblk = pair * 2 + sub
                        hh, half = divmod(cblk, 2)
                        for ts in range(4):
                            key = (hh, tj * 4 + ts)
                            en = encnt.pop(key) if half == 1 else encnt[key]
                            nc.tensor.transpose(
                                ptr[:, sub * TJ + ts * P:
                                    sub * TJ + (ts + 1) * P],
                                en[:, half * P:(half + 1) * P],
                                ident[:, :])

                        def job(cblk=cblk, ptr=ptr, sub=sub):
                            nc.any.tensor_copy(
                                enc[cblk][:, tj * TJ:(tj + 1) * TJ],
                                ptr[:, sub * TJ:(sub + 1) * TJ])
                        jobs.append(job)
                return jobs

            def outproj_jobs(tj):
                tsl = slice(tj * TJ, (tj + 1) * TJ)
                jobs = []
                for dt in range(DCH):
                    def job(dt=dt, use_act=False):
                        # po shares the "ptr" psum slots (2KB each): its
                        # slot-reuse waits resolve to already-emitted copies
                        po = ps.tile([P, TJ], f32, tag="ptr", bufs=2,
                                     name="po")
                        for c in range(NHC):
                            nc.tensor.matmul(
                                po[:, :],
                                wo_sb[dt][:, c * P:(c + 1) * P],
                                enc[c][:, tsl],
                                start=(c == 0), stop=(c == NHC - 1),
                            )
                        oc = sb.tile([P, TJ], bf16, tag="outc", bufs=3,
                                     name="oc")
                        cp = nc.any.tensor_copy if use_act \
                            else nc.vector.tensor_copy
                        cp(oc[:, :], po[:, :])
                        nc.sync.dma_start(out_e.ap()[dt][:, tsl], oc[:, :])
                    jobs.append(job)
                return jobs

            # schedule: [attn tj (interleaving tj-1's deferred ACT copies and
            # out-proj dt-groups into the st-loop, and the next unit's first
            # QK steps into the final PV flush)] [transposes tj] ...
            units = [(hh, tj) for tj in range(NTJ)
                     for hh in range(HEADS_PER_CORE)]
            act_pending = []
            pe_pending = []
            pre = []
            for ui, (hh, tj) in enumerate(units):
                nxt = units[ui + 1] if ui + 1 < len(units) else None
                pre_next = []
                tails = []
                if nxt is not None:
                    nhh, ntj = nxt
                    for st in s_tiles[ntj][:2]:
                        def tail(nhh=nhh, ntj=ntj, st=st):
                            pre_next.append(qk_step(nhh, ntj, st))
                        tails.append(tail)
                attention(hh, tj, act_pending if hh == 0 else None,
                          pe_pending, pe_gate=5 if hh == 0 else 0,
                          pre=pre, tail_jobs=tails)
                pre = pre_next
                if hh == HEADS_PER_CORE - 1:
                    for job in act_pending:  # copies not yet interleaved
                        job()
                    for i, job in enumerate(pe_pending):
                        job(use_act=(i % 2 == 0))  # drain on both engines
                    act_pending = transposes(tj)
                    pe_pending = outproj_jobs(tj)
            for job in act_pending:
                job()
            for i, job in enumerate(pe_pending):
                job(use_act=(i % 2 == 0))

    nc.compile()
    return nc


def _prep_core_inputs(core, x, wq_eff, wk_eff, wv_eff, wo_eff, cos, sin,
                      attn_mask, bias_list):
    b, j = divmod(core, 4)
    nh0 = j * HEADS_PER_CORE * H  # first flattened q/o column of this core

    xT = np.ascontiguousarray(x[b].T).reshape(DCH, P, T)

    wq4 = wq_eff.reshape(DCH, P, NH // P, P)
    wq = np.stack([
        np.ascontiguousarray(wq4[:, :, nh0 // P + n, :].transpose(1, 0, 2)
                             ).reshape(P, DCH * P)
        for n in range(NHC)
    ])
    wk4 = wk_eff.reshape(DCH, P, 2, P)
    wk = np.stack([
        np.ascontiguousarray(wk4[:, :, jh, :].transpose(1, 0, 2)
                             ).reshape(P, DCH * P)
        for jh in range(2)
    ])
    wv = wv_eff.reshape(DCH, P, H)
    woc = wo_eff[nh0:nh0 + HEADS_PER_CORE * H, :].reshape(NHC, P, DCH, P)
    wo = np.stack([
        np.ascontiguousarray(woc[:, :, dt, :].transpose(1, 0, 2)
                             ).reshape(P, NHC * P)
        for dt in range(DCH)
    ])

    cb, sbn = cos[b], sin[b]
    scale = float(H) ** -0.5
    csq = np.stack([cb * scale, sbn * scale])
    csk = np.stack([cb, sbn])

    m = np.asarray(attn_mask)[b, 0]  # [T(query), S(key)]
    if bias_list:
        btiles = []
        for (tj, st, mc) in bias_list:
            sub = m[tj * TJ + mc * P:tj * TJ + (mc + 1) * P,
                    st * P:(st + 1) * P].T  # [s, t]
            btiles.append(np.where(sub, np.float32(0.0), np.float32(BIG_NEG)))
        bias = np.stack(btiles)
    else:
        bias = np.zeros((1, P, P), np.float32)

    cast = lambda a: np.ascontiguousarray(a).astype(_BF16)
    return {
        "xT": cast(xT), "wq": cast(wq), "wk": cast(wk), "wv": cast(wv),
        "wo": cast(wo), "csq": cast(csq), "csk": cast(csk), "bias": cast(bias),
        "ident": np.eye(P, dtype=np.float32).astype(_BF16),
    }


def kernel(x, positions, attn_mask, decode, q_w, q_lora_a, q_lora_b,
           kv_w, kv_lora_a, kv_lora_b, out_w, out_lora_a, out_lora_b):
    global LAST_EXEC_TIME_NS
    from concourse.bass_utils import run_bass_kernel_spmd

    x = np.asarray(x, np.float32)
    positions = np.asarray(positions)
    attn_mask = np.asarray(attn_mask)

    wq_eff, wk_eff, wv_eff, wo_eff = _fold_weights(
        np.asarray(q_w, np.float32), np.asarray(q_lora_a, np.float32),
        np.asarray(q_lora_b, np.float32), np.asarray(kv_w, np.float32),
        np.asarray(kv_lora_a, np.float32), np.asarray(kv_lora_b, np.float32),
        np.asarray(out_w, np.float32), np.asarray(out_lora_a, np.float32),
        np.asarray(out_lora_b, np.float32))

    cos, sin = [], []
    for b in range(B):
        c_, s_ = _rope_tables(positions[b])
        cos.append(c_)
        sin.append(s_)

    s_tiles, lo_cols, bias_list = _classify_mask(attn_mask)
    nc = _build_program(s_tiles, lo_cols, bias_list)

    in_maps = [
        _prep_core_inputs(core, x, wq_eff, wk_eff, wv_eff, wo_eff, cos, sin,
                          attn_mask, bias_list)
        for core in range(N_CORES)
    ]

    trace = os.environ.get("KERNEL_PROFILE", "0") == "1"
    if trace:
        try:
            import antenv.axon_hooks  # noqa: F401
        except ImportError:
            trace = False
    res = run_bass_kernel_spmd(nc, in_maps, core_ids=list(range(N_CORES)),
                               trace=trace,
                               tmpdir=os.environ.get("KERNEL_TMPDIR"))
    LAST_EXEC_TIME_NS = res.exec_time_ns

    out = np.zeros((B, T, D), np.float32)
    for core in range(N_CORES):
        b = core // 4
        part = res.results[core]["out"].reshape(D, T)  # [d, t] bf16
        out[b] += part.T.astype(np.float32)
    return out


# revision 48
# speedup vs baseline: 1.0048x; 1.0048x over previous
"""Distributed Trainium2 kernel for LoRA-fused GQA attention.

Sharding: 8 cores = 2 (batch) x 4 (head-pairs). Core 4*b+j computes batch b,
heads {2j, 2j+1} end-to-end (q-proj, RoPE, causal attention vs the shared
KV head, partial out-proj); the host sums the 4 partial out-projections per
batch. LoRA adapters are folded into effective projection weights on the host
(exact algebraic rewrite; the einsum's repeated head index is a diagonal).

v2: the PE p-state ramp only reaches full clock (2.4 GHz) after ~3us of
gap-free execution; any short stall drops it to ~2.0 GHz. This version keeps
the PE stream continuous: the PV matmul is flipped to produce enc^T [t, H+1]
with a ones-column appended to V so the softmax denominator arrives as a free
extra output column (no separate l-matmuls, no PSUM-serialized l tile, no
DRAM-broadcast normalize — just a per-partition reciprocal + tensor_scalar).
enc^T is PE-transposed back to [nh, t] for the out-projection. All matmuls
use uniform 128x128 stationary tiles.

Self-contained: hardcodes all shapes from the problem spec.
"""

import os

import numpy as np
import ml_dtypes

B, T, D, N, K, H, L = 2, 2048, 2048, 8, 1, 256, 16
LORA_SCALE = 16.0 / 16
BIG_NEG = -2.3819763e38
N_CORES = 8
P = 128
NH = N * H  # 2048
TJ = 512  # t-chunk width (free dim of logits tiles)
NTJ = T // TJ  # 4
NST = T // P  # 16 s-tiles
DCH = D // P  # 16 contraction chunks
HEADS_PER_CORE = 2
NHC = HEADS_PER_CORE * H // P  # 4 q/enc row-tiles of 128 per core

LAST_EXEC_TIME_NS = None

_BF16 = ml_dtypes.bfloat16


def _fold_weights(q_w, q_lora_a, q_lora_b, kv_w, kv_lora_a, kv_lora_b,
                  out_w, out_lora_a, out_lora_b):
    """Fold LoRA into effective dense weights (all float32 math)."""
    # q: [N,D,H] + a[N,D,L] @ diag_b[N,L,H] -> Wq_eff [D, N*H]
    bd = np.stack([q_lora_b[n, :, n, :] for n in range(N)])  # [N,L,H]
    wq = q_w + np.einsum("ndl,nlh->ndh", q_lora_a, bd) * LORA_SCALE
    wq_eff = wq.transpose(1, 0, 2).reshape(D, NH).astype(np.float32)
    # kv: [2,K,D,H], K=1
    kvb = kv_lora_b[:, 0, :, 0, :]  # [2,L,H]
    wkv = kv_w[:, 0] + np.einsum("idl,ilh->idh", kv_lora_a[:, 0], kvb) * LORA_SCALE
    wk_eff = wkv[0].astype(np.float32)  # [D,H]
    wv_eff = wkv[1].astype(np.float32)  # [D,H]
    # out: [N,H,D] + diag_a[N,H,L] @ b[N,L,D] -> Wo_eff [N*H, D]
    ad = np.stack([out_lora_a[n, :, n, :] for n in range(N)])  # [N,H,L]
    wo = out_w + np.einsum("nhl,nld->nhd", ad, out_lora_b) * LORA_SCALE
    wo_eff = wo.reshape(NH, D).astype(np.float32)
    return wq_eff, wk_eff, wv_eff, wo_eff


def _rope_tables(positions_b):
    """cos/sin [P, T] f32 for one batch row of positions."""
    freq_exp = (2.0 / H) * np.arange(H // 2, dtype=np.float32)
    timescale = (10000.0 ** freq_exp).astype(np.float32)  # [128]
    rad = positions_b.astype(np.float32)[None, :] / timescale[:, None]  # [128,T]
    return np.cos(rad).astype(np.float32), np.sin(rad).astype(np.float32)


def _classify_mask(attn_mask):
    """Tile-classify the (shared-program) mask.

    Returns (s_tiles, lo_cols, bias_list):
      s_tiles[tj]: ordered 128-wide s-tile indices to process for t-chunk tj
      lo_cols[(tj, st)]: leading all-false column count in the [128 s, TJ t]
        logits tile (those t-columns are skipped: exp never runs there and
        the flipped PV never reads them)
      bias_list: (tj, st, mc) 128-col sub-blocks needing an additive bias
    Inclusion is the union over both batches so the SPMD program is
    identical on every core; per-core bias data covers the rest.
    """
    m = np.asarray(attn_mask)[:, 0]  # [B, T(query), S(key)]
    s_tiles = []
    lo_cols = {}
    bias_list = []
    for tj in range(NTJ):
        tiles = []
        for st in range(NST):
            sk = slice(st * P, (st + 1) * P)
            subs = [m[:, tj * TJ + mc * P:tj * TJ + (mc + 1) * P, sk]
                    for mc in range(TJ // P)]
            if not any(s.any() for s in subs):
                continue
            tiles.append(st)
            lo = 0
            while lo < len(subs) and not subs[lo].any():
                lo += 1
            lo_cols[(tj, st)] = lo * P
            for mc in range(lo, len(subs)):
                if not subs[mc].all():
                    bias_list.append((tj, st, mc))
        s_tiles.append(tiles)
    return s_tiles, lo_cols, bias_list


def _build_program(s_tiles, lo_cols, bias_list):
    """Build + compile the single SPMD Bass program."""
    from contextlib import ExitStack

    import concourse.bass as bass  # noqa: F401
    import concourse.tile as tile
    from concourse import bacc, mybir

    bf16 = mybir.dt.bfloat16
    f32 = mybir.dt.float32
    AF = mybir.ActivationFunctionType

    nc = bacc.Bacc("TRN2", target_bir_lowering=False, debug=False,
                   num_devices=N_CORES)

    xT_e = nc.dram_tensor("xT", [DCH, P, T], bf16, kind="ExternalInput")
    wq_e = nc.dram_tensor("wq", [NHC, P, DCH * P], bf16, kind="ExternalInput")
    wk_e = nc.dram_tensor("wk", [2, P, DCH * P], bf16, kind="ExternalInput")
    wv_e = nc.dram_tensor("wv", [DCH, P, H], bf16, kind="ExternalInput")
    wo_e = nc.dram_tensor("wo", [DCH, P, NHC * P], bf16, kind="ExternalInput")
    csq_e = nc.dram_tensor("csq", [2, P, T], bf16, kind="ExternalInput")
    csk_e = nc.dram_tensor("csk", [2, P, T], bf16, kind="ExternalInput")
    id_e = nc.dram_tensor("ident", [P, P], bf16, kind="ExternalInput")
    nbias = max(1, len(bias_list))
    bias_e = nc.dram_tensor("bias", [nbias, P, P], bf16, kind="ExternalInput")
    out_e = nc.dram_tensor("out", [DCH, P, T], bf16, kind="ExternalOutput")

    bias_idx = {k: i for i, k in enumerate(bias_list)}

    with tile.TileContext(nc) as tc:
        with ExitStack() as ctx:
            sb = ctx.enter_context(tc.tile_pool(name="sb", bufs=1))
            ps = ctx.enter_context(tc.tile_pool(name="ps", bufs=1,
                                                space="PSUM"))

            warm_w = sb.tile([P, P], bf16, tag="warm", bufs=2, name="warmw")
            nc.vector.memset(warm_w[:, :], 0.0)
            warm_src = sb.tile([P, TJ], bf16, tag="warm2", bufs=1,
                               name="warms")
            nc.vector.memset(warm_src[:, :], 0.0)

            # ---- loads, in consumption order. wave1 of the q-projection
            # only uses head-block 0, so load wq0 + the first x chunks first
            # and start the real matmuls ~4us sooner.
            wq_sb = []
            for n in range(NHC):
                t_ = sb.tile([P, DCH * P], bf16, tag="wq", bufs=NHC,
                             name=f"wqs{n}")
                wq_sb.append(t_)
            xt = []
            for c in range(DCH):
                t_ = sb.tile([P, T], bf16, tag="xs", bufs=DCH, name=f"xt{c}")
                xt.append(t_)
            nc.sync.dma_start(wq_sb[0][:, :], wq_e.ap()[0])
            for c in range(2):
                nc.sync.dma_start(xt[c][:, :], xT_e.ap()[c])
            for n in range(1, NHC):
                nc.sync.dma_start(wq_sb[n][:, :], wq_e.ap()[n])
            wk_sb = []
            for j in range(2):
                t_ = sb.tile([P, DCH * P], bf16, tag="wk", bufs=2,
                             name=f"wks{j}")
                nc.sync.dma_start(t_[:, :], wk_e.ap()[j])
                wk_sb.append(t_)
            for c in range(2, DCH):
                nc.sync.dma_start(xt[c][:, :], xT_e.ap()[c])
            csq = []
            csk = []
            for i in range(2):
                t_ = sb.tile([P, T], bf16, tag="cs", bufs=4, name=f"csq{i}")
                nc.sync.dma_start(t_[:, :], csq_e.ap()[i])
                csq.append(t_)
            for i in range(2):
                t_ = sb.tile([P, T], bf16, tag="cs", bufs=4, name=f"csk{i}")
                nc.sync.dma_start(t_[:, :], csk_e.ap()[i])
                csk.append(t_)
            # v tiles carry a ones-column at [:, H] so the flipped PV matmul
            # emits the softmax denominator as output column H
            v_sb = []
            for st in range(NST):
                vt = sb.tile([P, H + 1], bf16, tag="v", bufs=NST,
                             name=f"v{st}")
                nc.vector.memset(vt[:, H:H + 1], 1.0)
                v_sb.append(vt)
            wv_sb = []
            for c in range(DCH):
                t_ = sb.tile([P, H], bf16, tag="wv", bufs=DCH, name=f"wv{c}")
                nc.sync.dma_start(t_[:, :], wv_e.ap()[c])
                wv_sb.append(t_)
            wo_sb = []
            for dt in range(DCH):
                t_ = sb.tile([P, NHC * P], bf16, tag="wo", bufs=DCH,
                             name=f"wos{dt}")
                nc.sync.dma_start(t_[:, :], wo_e.ap()[dt])
                wo_sb.append(t_)
            ident = sb.tile([P, P], bf16, tag="ident", bufs=1, name="ident")
            nc.sync.dma_start(ident[:, :], id_e.ap())
            bias_sb = {}
            for key in bias_list:
                tj, st, mc = key
                t_ = sb.tile([P, P], bf16, tag="bias", bufs=nbias,
                             name=f"bias{tj}_{st}_{mc}")
                nc.sync.dma_start(t_[:, :], bias_e.ap()[bias_idx[key]])
                bias_sb[key] = t_

            # PE warm-up during the initial DMA wait: throwaway matmuls with
            # the same 128x128 stationary shape as everything else, so the
            # p-state ramp opens before real work lands.
            def filler():
                wps = ps.tile([P, TJ], f32, tag="ptr", bufs=2, name="wps")
                nc.tensor.matmul(wps[:, :], warm_w[:, :], warm_src[:, :],
                                 start=True, stop=True)

            for i in range(24):
                filler()

            def rope_pair(dst0, dst1, src0, src1, cs):
                """dst0 = s0*cos - s1*sin ; dst1 = s1*cos + s0*sin."""
                cos_t, sin_t = cs
                a = sb.tile([P, T], bf16, tag="qk", bufs=10, name="ropeA")
                nc.vector.tensor_mul(a[:, :], src0[:, :], cos_t[:, :])
                bmul = sb.tile([P, T], bf16, tag="qk", bufs=10, name="ropeB")
                nc.vector.tensor_mul(bmul[:, :], src1[:, :], sin_t[:, :])
                c2 = sb.tile([P, T], bf16, tag="qk", bufs=10, name="ropeC")
                nc.vector.tensor_mul(c2[:, :], src1[:, :], cos_t[:, :])
                d2 = sb.tile([P, T], bf16, tag="qk", bufs=10, name="ropeD")
                nc.vector.tensor_mul(d2[:, :], src0[:, :], sin_t[:, :])
                nc.vector.tensor_sub(dst0[:, :], a[:, :], bmul[:, :])
                nc.vector.tensor_add(dst1[:, :], c2[:, :], d2[:, :])

            # ---- stage A: q projection (2 heads) + RoPE ----
            # psum group tags: "pse" x4 + "lg" x2 rotate through projections
            atags = [("pse", 2), ("pse", 2), ("lg", 4), ("lg", 4),
                     ("lg", 4), ("lg", 4)]
            qraw = [sb.tile([P, T], bf16, tag="qk", bufs=10, name=f"qraw{n}")
                    for n in range(NHC)]
            groups = [(n, t4) for n in range(NHC) for t4 in range(T // 512)]
            # wave 1: 4 open accumulation groups on the "pse" banks,
            # interleaved chunk-major so the PE consumes each x chunk the
            # moment its DMA lands; fillers (on "lg") pad the per-chunk PE
            # time to the DMA cadence.
            wave1 = groups[:4]
            pq1 = {}
            for (n, t4) in wave1:
                pq1[(n, t4)] = ps.tile([P, 512], f32, tag="lg", bufs=4,
                                       name="pq")
            for c in range(DCH):
                for (n, t4) in wave1:
                    nc.tensor.matmul(
                        pq1[(n, t4)][:, :],
                        wq_sb[n][:, c * P:(c + 1) * P],
                        xt[c][:, t4 * 512:(t4 + 1) * 512],
                        start=(c == 0), stop=(c == DCH - 1),
                    )
                filler()
                filler()
            for (n, t4) in wave1:
                nc.any.tensor_copy(qraw[n][:, t4 * 512:(t4 + 1) * 512],
                                   pq1[(n, t4)][:, :])
            # wave 2: x is resident by now; plain group-major
            gi = 0
            for (n, t4) in groups[4:]:
                tg, nb = atags[gi % len(atags)]
                gi += 1
                pq = ps.tile([P, 512], f32, tag=tg, bufs=nb, name="pq")
                for c in range(DCH):
                    nc.tensor.matmul(
                        pq[:, :],
                        wq_sb[n][:, c * P:(c + 1) * P],
                        xt[c][:, t4 * 512:(t4 + 1) * 512],
                        start=(c == 0), stop=(c == DCH - 1),
                    )
                nc.any.tensor_copy(qraw[n][:, t4 * 512:(t4 + 1) * 512],
                                   pq[:, :])
            q = []
            for hh in range(HEADS_PER_CORE):
                q0 = sb.tile([P, T], bf16, tag="qk", bufs=10, name=f"q{2*hh}")
                q1 = sb.tile([P, T], bf16, tag="qk", bufs=10, name=f"q{2*hh+1}")
                rope_pair(q0, q1, qraw[2 * hh], qraw[2 * hh + 1], csq)
                q.extend([q0, q1])

            # ---- stage B: k/v projection (replicated KV head) + RoPE ----
            kraw = []
            for j in range(2):
                kn = sb.tile([P, T], bf16, tag="qk", bufs=10, name=f"kraw{j}")
                for t4 in range(T // 512):
                    tg, nb = atags[gi % len(atags)]
                    gi += 1
                    pk = ps.tile([P, 512], f32, tag=tg, bufs=nb, name="pk")
                    for c in range(DCH):
                        nc.tensor.matmul(
                            pk[:, :],
                            wk_sb[j][:, c * P:(c + 1) * P],
                            xt[c][:, t4 * 512:(t4 + 1) * 512],
                            start=(c == 0), stop=(c == DCH - 1),
                        )
                    nc.any.tensor_copy(kn[:, t4 * 512:(t4 + 1) * 512], pk[:, :])
                kraw.append(kn)
            k0 = sb.tile([P, T], bf16, tag="qk", bufs=10, name="k0")
            k1 = sb.tile([P, T], bf16, tag="qk", bufs=10, name="k1")
            # k rope column-chunked: the first QK matmuls gate on the low
            # key columns, so finish those first
            kta = sb.tile([P, T], bf16, tag="qk", bufs=10, name="kta")
            ktb = sb.tile([P, T], bf16, tag="qk", bufs=10, name="ktb")
            ktc = sb.tile([P, T], bf16, tag="qk", bufs=10, name="ktc")
            ktd = sb.tile([P, T], bf16, tag="qk", bufs=10, name="ktd")
            for (x0, x1) in [(0, 512), (512, T)]:
                sl = slice(x0, x1)
                nc.vector.tensor_mul(kta[:, sl], kraw[0][:, sl], csk[0][:, sl])
                nc.vector.tensor_mul(ktb[:, sl], kraw[1][:, sl], csk[1][:, sl])
                nc.vector.tensor_sub(k0[:, sl], kta[:, sl], ktb[:, sl])
                nc.vector.tensor_mul(ktc[:, sl], kraw[1][:, sl], csk[0][:, sl])
                nc.vector.tensor_mul(ktd[:, sl], kraw[0][:, sl], csk[1][:, sl])
                nc.vector.tensor_add(k1[:, sl], ktc[:, sl], ktd[:, sl])

            for st in range(NST):
                tg, nb = atags[gi % len(atags)]
                gi += 1
                pv = ps.tile([P, H], f32, tag=tg, bufs=nb, name="pv")
                for c in range(DCH):
                    nc.tensor.matmul(
                        pv[:, :],
                        xt[c][:, st * P:(st + 1) * P],
                        wv_sb[c][:, :],
                        start=(c == 0), stop=(c == DCH - 1),
                    )
                # explicit DVE: ACT's in-order queue must stay clear for exp
                nc.vector.tensor_copy(v_sb[st][:, 0:H], pv[:, :])

            # ---- stage C: attention per (head, t-chunk) ----
            # QK unchanged: lg [128s, TJ t]. PV flipped: for each 128-wide
            # t-slice, enc_T[128t, H+1] = sum_st p[st][:, t-slice]^T @ [v|1].
            # Column H is the softmax denominator. Normalize with a
            # per-partition reciprocal; PE-transpose back to [nh, t] blocks
            # one t-chunk later; out-proj consumes the reassembled enc.
            enc = []
            for n in range(NHC):
                # tag "xs": reuse the xT slots (dead after the v projection)
                en = sb.tile([P, T], bf16, tag="xs", bufs=DCH, name=f"enc{n}")
                enc.append(en)
            # normalized enc_T slices [128t, H] per (hh, local ts); live
            # until transposed during the next t-chunk
            encnt = {}

            def qk_step(hh, tj, st):
                """Emit QK + bias + exp for one s-tile; return pv args."""
                tsl0 = tj * TJ
                ssl = slice(st * P, (st + 1) * P)
                lo = lo_cols[(tj, st)]
                lg = ps.tile([P, TJ], f32, tag="lg", bufs=4, name="lg")
                nc.tensor.matmul(lg[:, lo:], k0[:, ssl],
                                 q[2 * hh][:, tsl0 + lo:tsl0 + TJ],
                                 start=True, stop=False)
                nc.tensor.matmul(lg[:, lo:], k1[:, ssl],
                                 q[2 * hh + 1][:, tsl0 + lo:tsl0 + TJ],
                                 start=False, stop=True)
                for mc in range(lo // P, TJ // P):
                    key = (tj, st, mc)
                    if key in bias_sb:
                        msl = slice(mc * P, (mc + 1) * P)
                        nc.vector.tensor_add(lg[:, msl], lg[:, msl],
                                             bias_sb[key][:, :])
                p_sb = sb.tile([P, TJ], bf16, tag="p", bufs=17, name="psb")
                nc.scalar.activation(p_sb[:, lo:], lg[:, lo:], AF.Exp)
                return (st, lo, p_sb)

            def attention(hh, tj, act_jobs, pe_jobs=None, pe_gate=0,
                          pre=None, tail_jobs=None):
                """Two-phase unit: t-slices are processed in pairs so only
                2 encT banks are live at once (pse bufs=2), freeing lg to 4
                banks. Pair 1 replays the cached p tiles as a ready-PV
                backlog, so its new QK/exp latencies are fully hidden.

                pre: pv args for this unit's first tiles (QK/exp emitted by
                the previous unit). tail_jobs: closures emitting the NEXT
                unit's first QK steps, interleaved into this unit's final PV
                flush so the in-order PE never drains on the last exp."""
                tiles = s_tiles[tj]
                nts = TJ // P  # 4 local t-slices
                if not tiles:
                    for ts in range(nts):
                        en = sb.tile([P, H], bf16, tag="ent", bufs=8,
                                     name="encnt")
                        nc.vector.memset(en[:, :], 0.0)
                        encnt[(hh, tj * 4 + ts)] = en
                    return
                first_for = {}
                last_for = {}
                for st in tiles:
                    for ts in range(nts):
                        if (tj * 4 + ts) >= st:
                            first_for.setdefault(ts, st)
                            last_for[ts] = st
                alt = [0]
                p_cache = {}
                for (st, lo, p_sb) in (pre or []):
                    p_cache[st] = (lo, p_sb)
                tails = list(tail_jobs or [])

                def normalize(ts):
                    en = sb.tile([P, H], bf16, tag="ent", bufs=8,
                                 name="encnt")
                    if ts not in first_for:
                        nc.vector.memset(en[:, :], 0.0)
                        encnt[(hh, tj * 4 + ts)] = en
                        return
                    rec = sb.tile([P, 1], f32, tag="rec", bufs=4, name="rec")
                    nc.vector.reciprocal_approx_fast(
                        out=rec[:, :], in_=encT[ts][:, H:H + 1])
                    nc.vector.tensor_scalar_mul(en[:, :],
                                                encT[ts][:, 0:H], rec[:, :])
                    encnt[(hh, tj * 4 + ts)] = en

                for pair in range(2):
                    ts_list = [2 * pair, 2 * pair + 1]
                    ts_exist = [ts for ts in ts_list if ts in first_for]
                    if not ts_exist:
                        for ts in ts_list:
                            normalize(ts)
                        continue
                    encT = {}
                    for ts in ts_exist:
                        encT[ts] = ps.tile([P, TJ], f32, tag="pse", bufs=2,
                                           name="encT")

                    def emit_pv(st, lo, p_sb):
                        for ts in ts_exist:
                            tt = tj * 4 + ts
                            if tt < st:
                                continue
                            nc.tensor.matmul(
                                encT[ts][:, 0:H + 1],
                                p_sb[:, ts * P:(ts + 1) * P],
                                v_sb[st][:, :],
                                start=(st == first_for[ts]),
                                stop=(st == last_for[ts]),
                            )

                    tt_hi = tj * 4 + ts_exist[-1]
                    needed = [st for st in tiles if st <= tt_hi]
                    backlog = [st for st in needed if st in p_cache]
                    new_tiles = [st for st in needed if st not in p_cache]
                    pv_q = []
                    for idx, st in enumerate(new_tiles):
                        if pe_jobs and (pair == 1 or idx >= pe_gate):
                            # pair1: alternate engines to halve queue bursts
                            alt[0] += 1
                            pe_jobs.pop(0)(use_act=(pair == 1
                                                    and alt[0] % 2 == 0))
                        p_cache[st] = qk_step(hh, tj, st)[1:]
                        if act_jobs:
                            act_jobs.pop(0)()  # deferred ACT enc copy
                        if backlog:  # ready PVs hide the new exp latency
                            bst = backlog.pop(0)
                            emit_pv(bst, *p_cache[bst])
                        pv_q.append(st)
                        if len(pv_q) > 2:
                            bst = pv_q.pop(0)
                            emit_pv(bst, *p_cache[bst])
                    for st in backlog:
                        if pe_jobs and pair == 1:
                            alt[0] += 1
                            pe_jobs.pop(0)(use_act=(alt[0] % 2 == 0))
                        emit_pv(st, *p_cache[st])
                    last_flush = pair == 1 or not any(
                        ts in first_for for ts in (2, 3))
                    for st in pv_q:
                        if last_flush and tails:
                            tails.pop(0)()  # next unit's QK fills exp wait
                        emit_pv(st, *p_cache[st])
                    for ts in ts_list:
                        normalize(ts)
                for job in tails:
                    job()

            def transposes(tj):
                """PE-transpose enc_T(tj) into [nh, t] blocks; returns the
                deferred ACT copy closures (one per nh block)."""
                jobs = []
                for pair in range(NHC // 2):
                    ptr = ps.tile([P, 2 * TJ], bf16, tag="ptr", bufs=2,
                                  name="ptr")
                    for sub in range(2):
                        cblk = pair * 2 + sub
                        hh, half = divmod(cblk, 2)
                        for ts in range(4):
                            key = (hh, tj * 4 + ts)
                            en = encnt.pop(key) if half == 1 else encnt[key]
                            nc.tensor.transpose(
                                ptr[:, sub * TJ + ts * P:
                                    sub * TJ + (ts + 1) * P],
                                en[:, half * P:(half + 1) * P],
                                ident[:, :])

                        def job(cblk=cblk, ptr=ptr, sub=sub):
                            nc.any.tensor_copy(
                                enc[cblk][:, tj * TJ:(tj + 1) * TJ],
                                ptr[:, sub * TJ:(sub + 1) * TJ])
                        jobs.append(job)
                return jobs

            def outproj_jobs(tj):
                tsl = slice(tj * TJ, (tj + 1) * TJ)
                jobs = []
                for dt in range(DCH):
                    def job(dt=dt, use_act=False):
                        # po shares the "ptr" psum slots (2KB each): its
                        # slot-reuse waits resolve to already-emitted copies
                        po = ps.tile([P, TJ], f32, tag="ptr", bufs=2,
                                     name="po")
                        for c in range(NHC):
                            nc.tensor.matmul(
                                po[:, :],
                                wo_sb[dt][:, c * P:(c + 1) * P],
                                enc[c][:, tsl],
                                start=(c == 0), stop=(c == NHC - 1),
                            )
                        oc = sb.tile([P, TJ], bf16, tag="outc", bufs=3,
                                     name="oc")
                        cp = nc.any.tensor_copy if use_act \
                            else nc.vector.tensor_copy
                        cp(oc[:, :], po[:, :])
                        nc.sync.dma_start(out_e.ap()[dt][:, tsl], oc[:, :])
                    jobs.append(job)
                return jobs

            # schedule: [attn tj (interleaving tj-1's deferred ACT copies and
            # out-proj dt-groups into the st-loop, and the next unit's first
            # QK steps into the final PV flush)] [transposes tj] ...
            units = [(hh, tj) for tj in range(NTJ)
                     for hh in range(HEADS_PER_CORE)]
            act_pending = []
            pe_pending = []
            pre = []
            for ui, (hh, tj) in enumerate(units):
                nxt = units[ui + 1] if ui + 1 < len(units) else None
                pre_next = []
                tails = []
                if nxt is not None:
                    nhh, ntj = nxt
                    for st in s_tiles[ntj][:2]:
                        def tail(nhh=nhh, ntj=ntj, st=st):
                            pre_next.append(qk_step(nhh, ntj, st))
                        tails.append(tail)
                attention(hh, tj, act_pending if hh == 0 else None,
                          pe_pending, pe_gate=5 if hh == 0 else 0,
                          pre=pre, tail_jobs=tails)
                pre = pre_next
                if hh == HEADS_PER_CORE - 1:
                    for job in act_pending:  # copies not yet interleaved
                        job()
                    for i, job in enumerate(pe_pending):
                        job(use_act=(i % 2 == 0))  # drain on both engines
                    act_pending = transposes(tj)
                    pe_pending = outproj_jobs(tj)
            for job in act_pending:
                job()
            for i, job in enumerate(pe_pending):
                job(use_act=(i % 2 == 0))

    nc.compile()
    return nc


def _prep_core_inputs(core, x, wq_eff, wk_eff, wv_eff, wo_eff, cos, sin,
                      attn_mask, bias_list):
    b, j = divmod(core, 4)
    nh0 = j * HEADS_PER_CORE * H  # first flattened q/o column of this core

    xT = np.ascontiguousarray(x[b].T).reshape(DCH, P, T)

    wq4 = wq_eff.reshape(DCH, P, NH // P, P)
    wq = np.stack([
        np.ascontiguousarray(wq4[:, :, nh0 // P + n, :].transpose(1, 0, 2)
                             ).reshape(P, DCH * P)
        for n in range(NHC)
    ])
    wk4 = wk_eff.reshape(DCH, P, 2, P)
    wk = np.stack([
        np.ascontiguousarray(wk4[:, :, jh, :].transpose(1, 0, 2)
                             ).reshape(P, DCH * P)
        for jh in range(2)
    ])
    wv = wv_eff.reshape(DCH, P, H)
    woc = wo_eff[nh0:nh0 + HEADS_PER_CORE * H, :].reshape(NHC, P, DCH, P)
    wo = np.stack([
        np.ascontiguousarray(woc[:, :, dt, :].transpose(1, 0, 2)
                             ).reshape(P, NHC * P)
        for dt in range(DCH)
    ])

    cb, sbn = cos[b], sin[b]
    scale = float(H) ** -0.5
    csq = np.stack([cb * scale, sbn * scale])
    csk = np.stack([cb, sbn])

    m = np.asarray(attn_mask)[b, 0]  # [T(query), S(key)]
    if bias_list:
        btiles = []
        for (tj, st, mc) in bias_list:
            sub = m[tj * TJ + mc * P:tj * TJ + (mc + 1) * P,
                    st * P:(st + 1) * P].T  # [s, t]
            btiles.append(np.where(sub, np.float32(0.0), np.float32(BIG_NEG)))
        bias = np.stack(btiles)
    else:
        bias = np.zeros((1, P, P), np.float32)

    cast = lambda a: np.ascontiguousarray(a).astype(_BF16)
    return {
        "xT": cast(xT), "wq": cast(wq), "wk": cast(wk), "wv": cast(wv),
        "wo": cast(wo), "csq": cast(csq), "csk": cast(csk), "bias": cast(bias),
        "ident": np.eye(P, dtype=np.float32).astype(_BF16),
    }


def kernel(x, positions, attn_mask, decode, q_w, q_lora_a, q_lora_b,
           kv_w, kv_lora_a, kv_lora_b, out_w, out_lora_a, out_lora_b):
    global LAST_EXEC_TIME_NS
    from concourse.bass_utils import run_bass_kernel_spmd

    x = np.asarray(x, np.float32)
    positions = np.asarray(positions)
    attn_mask = np.asarray(attn_mask)

    wq_eff, wk_eff, wv_eff, wo_eff = _fold_weights(
        np.asarray(q_w, np.float32), np.asarray(q_lora_a, np.float32),
        np.asarray(q_lora_b, np.float32), np.asarray(kv_w, np.float32),
        np.asarray(kv_lora_a, np.float32), np.asarray(kv_lora_b, np.float32),
        np.asarray(out_w, np.float32), np.asarray(out_lora_a, np.float32),
        np.asarray(out_lora_b, np.float32))

    cos, sin = [], []
    for b in range(B):
        c_, s_ = _rope_tables(positions[b])
        cos.append(c_)
        sin.append(s_)

    s_tiles, lo_cols, bias_list = _classify_mask(attn_mask)
    nc = _build_program(s_tiles, lo_cols, bias_list)

    in_maps = [
        _prep_core_inputs(core, x, wq_eff, wk_eff, wv_eff, wo_eff, cos, sin,
                          attn_mask, bias_list)
        for core in range(N_CORES)
    ]

    trace = os.environ.get("KERNEL_PROFILE", "0") == "1"
    if trace:
        try:
            import antenv.axon_hooks  # noqa: F401
        except ImportError:
            trace = False
    res = run_bass_kernel_spmd(nc, in_maps, core_ids=list(range(N_CORES)),
                               trace=trace,
                               tmpdir=os.environ.get("KERNEL_TMPDIR"))
    LAST_EXEC_TIME_NS = res.exec_time_ns

    out = np.zeros((B, T, D), np.float32)
    for core in range(N_CORES):
        b = core // 4
        part = res.results[core]["out"].reshape(D, T)  # [d, t] bf16
        out[b] += part.T.astype(np.float32)
    return out
